# revision 10
# baseline (speedup 1.0000x reference)
"""Bass/Trainium2 kernel for BaseWindowAttention (8x8 windows, 8 heads, dim 256).

Data-parallel over 8 NeuronCores: each core processes one (b, l) image of
[128, 128, 256]. Fully fused on-device pipeline: qkv projection -> windowed
attention (64-token windows) -> output projection.

Layout strategy per core:
  - tokens are processed in "groups" of 512 = 4 window-pairs (wp = 2 adjacent
    8x8 windows = 128 tokens, partition order (w, r, c)).
  - x is pre-transposed on host to [32 groups, 2 ch-half, 128 ch, 512 tok] so
    the contraction dim (channels) lands on SBUF partitions with zero on-chip
    transposes for the projection stage.
  - qT/kT live as [outch(4 heads x 32d) partitions, tok free] -> attention
    score matmuls S^T = kT^T @ qT slice straight out of SBUF via
    tile_position packing (K=32, M=64, N=64).
  - E = exp(S^T) * exp(bias^T) with the 64x64 relative-position bias table
    applied as a precomputed multiplicative table.
  - AV uses E as the stationary operand and V-natural as moving, with a ones
    column appended to V so each window-head matmul also emits the softmax
    denominator. Output lands token-major -> normalize is a native
    per-partition broadcast multiply.
  - O is PE-transposed back to [ch, tok] for the out projection; the result
    is written straight to the natural output layout by DMA.
"""

import os
import numpy as np
import ml_dtypes

import concourse.bass as bass
import concourse.bacc as bacc
import concourse.mybir as mybir
import concourse.tile as tile
from concourse.bass_utils import run_bass_kernel_spmd
from contextlib import ExitStack

F32 = mybir.dt.float32
F32R = mybir.dt.float32r
BF16 = mybir.dt.bfloat16

WS = 8
HEADS = 8
HD = 32
DIM = 256
STRIDE = 2 * WS - 1
SCALE = HD ** -0.5
N_CORES = 8
NG_FULL = 32  # 512-token groups per core

BF = ml_dtypes.bfloat16


def _bias_table() -> np.ndarray:
    # bias[qi, kj] from the 15x15 pos table, same as the reference
    coords = np.array([[x, y] for x in range(WS) for y in range(WS)], dtype=np.int32)
    rel = coords[None, :, :] - coords[:, None, :] + (WS - 1)
    idx = rel[:, :, 0] * STRIDE + rel[:, :, 1]
    return np.clip(idx, 0, None).reshape(WS * WS, WS * WS)


def build(n_groups: int = NG_FULL):
    nc = bacc.Bacc("TRN2", target_bir_lowering=False, debug=False,
                   num_devices=N_CORES)

    xT = nc.dram_tensor("xT", [NG_FULL, 128, 1024], BF16, kind="ExternalInput")
    wt = nc.dram_tensor("wt", [128, 1536], BF16, kind="ExternalInput")
    wo = nc.dram_tensor("wo", [128, 512], BF16, kind="ExternalInput")
    eb = nc.dram_tensor("eb", [128, 512], BF16, kind="ExternalInput")
    idn = nc.dram_tensor("idn", [128, 128], BF16, kind="ExternalInput")
    # output in kernel visit order [g, outch-half(p), (oh, tok)]; host
    # inverse-permutes to the natural image layout
    out = nc.dram_tensor("out", [NG_FULL, 128, 1024], BF16,
                         kind="ExternalOutput")

    with tile.TileContext(nc) as tc, ExitStack() as ctx:
        consts = ctx.enter_context(tc.tile_pool(name="consts", bufs=1))
        wt_s = consts.tile([128, 1536], BF16)
        wo_s = consts.tile([128, 512], BF16)
        eb_s = consts.tile([128, 512], BF16)
        idn_s = consts.tile([128, 128], BF16)
        nc.sync.dma_start(wt_s, wt.ap())
        nc.sync.dma_start(wo_s, wo.ap())
        nc.sync.dma_start(eb_s, eb.ap())
        nc.sync.dma_start(idn_s, idn.ap())

        # sbuf pools
        xt_p = ctx.enter_context(tc.tile_pool(name="xt", bufs=4))
        qk_p = ctx.enter_context(tc.tile_pool(name="qksb", bufs=4))
        v_p = ctx.enter_context(tc.tile_pool(name="vsb", bufs=4))
        e_p = ctx.enter_context(tc.tile_pool(name="esb", bufs=8))
        onm_p = ctx.enter_context(tc.tile_pool(name="onm", bufs=6))
        rc_p = ctx.enter_context(tc.tile_pool(name="rc", bufs=6))
        ot_p = ctx.enter_context(tc.tile_pool(name="ot", bufs=3))
        ob_p = ctx.enter_context(tc.tile_pool(name="ob", bufs=4))

        # psum pools (8 banks of [128, 2KB] total)
        # qkv: [128,512] slots, 2 banks; sp4: 4 banks (one per PE row-group --
        # concurrent row-tiled matmuls must write different banks or the
        # device dies); tail: shared 1-bank slots for AV out / O-transpose /
        # out-proj
        qkv_ps = ctx.enter_context(tc.tile_pool(name="qkvps", bufs=2, space="PSUM"))
        sp_ps = ctx.enter_context(tc.tile_pool(name="spps", bufs=1, space="PSUM"))
        tail_ps = ctx.enter_context(tc.tile_pool(name="tailps", bufs=2, space="PSUM"))

        for g in range(n_groups):
            xt01 = xt_p.tile([128, 1024], BF16, tag="xt01")
            nc.sync.dma_start(xt01, xT.ap()[g])
            xtr = [xt01[:, 0:512], xt01[:, 512:1024]]
            wtr = wt_s

            # ---- qkv projections (transposed layout for q/k, natural for v)
            qt = qk_p.tile([128, 1024], BF16, tag="qt")
            kt = qk_p.tile([128, 1024], BF16, tag="kt")
            for mt in range(4):  # 0,1 -> q halves; 2,3 -> k halves
                mps = qkv_ps.tile([128, 512], F32, tag="qkvp")
                for kh in range(2):
                    nc.tensor.matmul(
                        mps,
                        wtr[:, 768 * kh + 128 * mt:768 * kh + 128 * mt + 128],
                        xtr[kh], start=(kh == 0), stop=(kh == 1))
                dst = qt if mt < 2 else kt
                dsl = dst[:, 512 * (mt % 2):512 * (mt % 2) + 512]
                nc.scalar.activation(dsl, mps,
                                     mybir.ActivationFunctionType.Copy)

            va = v_p.tile([128, 1056], BF16, tag="va")  # (t4, h8, 33)
            va_r = va.rearrange("p (t h c) -> p t h c", t=4, h=8, c=33)
            for vh in range(2):  # two tok-tile pairs
                vps = qkv_ps.tile([128, 512], F32, tag="qkvp")
                for t2 in range(2):
                    t = 2 * vh + t2
                    for kh in range(2):
                        nc.tensor.matmul(
                            vps[:, 256 * t2:256 * t2 + 256],
                            xtr[kh][:, 128 * t:128 * t + 128],
                            wtr[:, 768 * kh + 512:768 * kh + 768],
                            start=(kh == 0), stop=(kh == 1))
                vps_r = vps.rearrange("p (t h c) -> p t h c", t=2, h=8, c=32)
                nc.vector.tensor_copy(va_r[:, 2 * vh:2 * vh + 2, :, 0:32], vps_r)
            nc.gpsimd.memset(va_r[:, :, :, 32], 1.0)

            ot = ot_p.tile([128, 1024], BF16, tag="ot")  # (ch-half, tok 512)

            for j in range(4):
                # ---- attention scores S^T[(w,kj), (h,qi)] for window pair j
                # one psum BANK per PE row-group (h4): concurrent row-tiled
                # matmuls into one bank are fatal on TRN2
                sp = sp_ps.tile([128, 2048], F32, tag="sp")
                for h in range(HEADS):
                    hh, h4 = divmod(h, 4)
                    for w in range(2):
                        col = 512 * hh + 128 * j + 64 * w
                        nc.tensor.matmul(
                            sp[64 * w:64 * w + 64,
                               512 * h4 + 64 * hh:512 * h4 + 64 * hh + 64],
                            kt[32 * h4:32 * h4 + 32, col:col + 64],
                            qt[32 * h4:32 * h4 + 32, col:col + 64],
                            start=True, stop=True,
                            tile_position=(32 * h4, 64 * w))

                sp_r = sp.rearrange("p (h4 r) -> p h4 r", h4=4)[:, :, 0:128]
                eraw = e_p.tile([128, 512], BF16, tag="eraw")
                er_r = eraw.rearrange("p (h4 r) -> p h4 r", h4=4)
                nc.scalar.activation(er_r, sp_r,
                                     mybir.ActivationFunctionType.Exp)
                et = e_p.tile([128, 512], BF16, tag="et")
                nc.gpsimd.tensor_mul(et, eraw, eb_s)

                # ---- AV with ones-augmented V: O[(w,qi), (h, 32d+denom)]
                on = tail_ps.tile([128, 512], F32, tag="tail")
                for h in range(HEADS):
                    hh, h4 = divmod(h, 4)
                    ecol = 64 * (2 * h4 + hh)
                    for w in range(2):
                        nc.tensor.matmul(
                            on[64 * w:64 * w + 64, 33 * h:33 * h + 33],
                            et[64 * w:64 * w + 64, ecol:ecol + 64],
                            va_r[64 * w:64 * w + 64, j, h, :],
                            start=True, stop=True,
                            tile_position=(64 * w, 64 * w))

                on_r = on[:, 0:264].rearrange("p (h c) -> p h c", h=8, c=33)
                rc = rc_p.tile([128, 8], F32, tag="rc")
                nc.vector.reciprocal(rc, on_r[:, :, 32])
                onm = onm_p.tile([128, 256], BF16, tag="onm")
                onm_r = onm.rearrange("p (h c) -> p h c", h=8, c=32)
                in0, in1 = bass.broadcast_tensor_aps(
                    on_r[:, :, 0:32], rc.rearrange("p (h o) -> p h o", o=1))
                nc.vector.tensor_tensor(onm_r, in0, in1, op=mybir.AluOpType.mult)

                # ---- transpose O back to [ch, tok]
                otp = tail_ps.tile([128, 256], BF16, tag="tail")
                for ch_half in range(2):
                    nc.tensor.transpose(
                        otp[:, 128 * ch_half:128 * ch_half + 128],
                        onm[:, 128 * ch_half:128 * ch_half + 128],
                        idn_s)
                ot_r = ot.rearrange("p (chh tok) -> p chh tok", chh=2)
                otp_r = otp.rearrange("p (chh tok) -> p chh tok", chh=2)
                nc.vector.tensor_copy(ot_r[:, :, 128 * j:128 * j + 128], otp_r)

            # ---- output projection (weight-stationary, ch-major out), DMA out
            ot_r2 = ot.rearrange("p (chh tok) -> p chh tok", chh=2)
            ob = ob_p.tile([128, 1024], BF16, tag="ob")
            for oh in range(2):
                op = tail_ps.tile([128, 512], F32, tag="tail")
                for kh in range(2):
                    nc.tensor.matmul(
                        op,
                        wo_s[:, 256 * kh + 128 * oh:256 * kh + 128 * oh + 128],
                        ot_r2[:, kh, :],
                        start=(kh == 0), stop=(kh == 1))
                if oh == 0:
                    nc.scalar.activation(ob[:, 0:512], op,
                                         mybir.ActivationFunctionType.Copy)
                else:
                    nc.vector.tensor_copy(ob[:, 512:1024], op)
            nc.sync.dma_start(out.ap()[g], ob)

    nc.compile()
    return nc


def _host_prep(x, W_qkv, W_out, b_out, pos_emb):
    b, l, H, W, _ = x.shape
    # xT: per core [32, 2, 128, 512] channel-major, window-pair token order
    xr = x.reshape(b * l, 16, WS, 2, 4, 2, WS, 2, 128)
    # dims: [core, wr, r, half, j, w, c, kh, p]
    xt = np.ascontiguousarray(xr.transpose(0, 1, 3, 8, 7, 4, 5, 2, 6))
    # -> [core, wr, half, p, kh, j, w, r, c]
    xt = xt.reshape(b * l, NG_FULL, 128, 1024).astype(BF)

    wq = np.concatenate([W_qkv[:, :256] * SCALE, W_qkv[:, 256:]], axis=1)
    wt = np.ascontiguousarray(
        wq.reshape(2, 128, 768).transpose(1, 0, 2).reshape(128, 1536))

    wo = np.ascontiguousarray(
        W_out.reshape(2, 128, 256).transpose(1, 0, 2).reshape(128, 512)
    ).astype(BF)


    bias = pos_emb.reshape(-1)[_bias_table().reshape(-1)].reshape(64, 64)
    ebt = np.tile(np.exp(bias.T), (2, 8)).astype(BF)

    idn = np.eye(128, dtype=np.float32).astype(BF)
    return xt, wt.astype(BF), wo, ebt, idn


_NC_CACHE = {}


def kernel(x, W_qkv, W_out, b_out, pos_emb):
    x = np.asarray(x, dtype=np.float32)
    W_qkv = np.asarray(W_qkv, dtype=np.float32)
    W_out = np.asarray(W_out, dtype=np.float32)
    b_out = np.asarray(b_out, dtype=np.float32)
    pos_emb = np.asarray(pos_emb, dtype=np.float32)

    b, l, H, W, _ = x.shape
    xt, wt, wo, ebt, idn = _host_prep(x, W_qkv, W_out, b_out, pos_emb)

    if "nc" not in _NC_CACHE:
        _NC_CACHE["nc"] = build()
    nc = _NC_CACHE["nc"]

    in_maps = [
        {"xT": np.ascontiguousarray(xt[i]), "wt": wt, "wo": wo,
         "eb": ebt, "idn": idn}
        for i in range(N_CORES)
    ]
    res = run_bass_kernel_spmd(
        nc, in_maps, list(range(N_CORES)),
        trace=bool(int(os.environ.get("KERNEL_TRACE", "0"))))
    if res.exec_time_ns is not None:
        print(f"HW exec time: {res.exec_time_ns} ns")
    outs = np.stack([res.results[i]["out"] for i in range(N_CORES)])
    return (_unscramble(outs) + b_out).reshape(b, l, H, W, DIM)


def _unscramble(o):
    # [cores, 32(wr,half), 128(outch-in-half p), 1024(oh, j, w, r, c)]
    #   -> [cores, 128, 128, 256]
    n = o.shape[0]
    o = o.astype(np.float32).reshape(n, 16, 2, 128, 2, 4, 2, WS, WS)
    # dims: n, wr, half, p, oh, j, w, r, c
    o = o.transpose(0, 1, 7, 2, 5, 6, 8, 4, 3)  # n wr r half j w c oh p
    return np.ascontiguousarray(o).reshape(n, 128, 128, DIM)


if __name__ == "__main__":
    # quick smoke: run on hardware with random inputs
    rng = np.random.default_rng(0)
    x = rng.standard_normal((2, 4, 128, 128, 256), dtype=np.float32)
    W_qkv = rng.standard_normal((256, 768), dtype=np.float32) * DIM ** -0.5
    W_out = rng.standard_normal((256, 256), dtype=np.float32) * 256 ** -0.5
    b_out = rng.standard_normal(256, dtype=np.float32) * 0.02
    pos_emb = rng.standard_normal((15, 15), dtype=np.float32)
    o = kernel(x=x, W_qkv=W_qkv, W_out=W_out, b_out=b_out, pos_emb=pos_emb)
    print(o.shape, o.dtype)

